# revision 1
# baseline (speedup 1.0000x reference)
"""Trainium2 Bass kernel for nn_DecoderAttention3 (2-layer LSTM decoder with
attention + vocab projection), distributed over 8 NeuronCores.

Strategy:
  Kernel 1 (data-parallel over batch): each core runs the full 2-layer
    LSTM + masked attention + output linear for its 8-batch shard,
    producing x3T [512, 480] (transposed, t-major).
  Host: gathers x3T shards into the full [512, 3840] activations.
  Kernel 2 (tensor-parallel over vocab): each core computes logits for a
    4000-wide vocab slice over all 3840 (b, t) positions.

Precision: fp16 matmul operands (fp32 PSUM accumulation), f32r for the
softmax/exp chain, fp16 LSTM elementwise state; projection in fp16.
Mask uses -30000 (fp16-safe, exp underflows to exactly 0 in fp32).
"""
import sys
for p in ('/opt/trn_rl_repo', '/root/.axon_site/_ro/trn_rl_repo'):
    if p not in sys.path:
        sys.path.insert(0, p)

import numpy as np

import concourse.bass as bass
import concourse.tile as tile
from concourse import bacc, mybir

F32R = mybir.dt.float32r
F32 = mybir.dt.float32
F16 = mybir.dt.float16
AF = mybir.ActivationFunctionType
OP = mybir.AluOpType

B, T, S, H, V, L = 64, 60, 60, 512, 32000, 2
NCORES = 8
BL = B // NCORES          # 8 local batches per core
G4 = 4 * H                # 2048
NT = T * BL               # 480
NBT = B * T               # 3840
NV = V // NCORES          # 4000 vocab slice per core
NCH = 8
CH = NV // NCH            # 500
NEG = -30000.0


# ---------------------------------------------------------------------------
# kernel 1 builder
# ---------------------------------------------------------------------------
def _build_k1():
    nc = bacc.Bacc("TRN2", target_bir_lowering=False, debug=False, num_devices=NCORES)
    d = {}
    d["x1T"] = nc.dram_tensor("x1T", [H, NT], F16, kind="ExternalInput").ap()
    d["encT"] = nc.dram_tensor("encT", [H, NT], F16, kind="ExternalInput").ap()
    d["enc_s"] = nc.dram_tensor("enc_s", [S, BL, H], F32R, kind="ExternalInput").ap()
    d["mask"] = nc.dram_tensor("mask", [1, NT], F16, kind="ExternalInput").ap()
    d["wihT"] = nc.dram_tensor("wihT", [2, H, G4], F16, kind="ExternalInput").ap()
    d["whhT"] = nc.dram_tensor("whhT", [2, H, G4], F16, kind="ExternalInput").ap()
    d["linT"] = nc.dram_tensor("linT", [2, 2 * H, H], F16, kind="ExternalInput").ap()
    d["bsum"] = nc.dram_tensor("bsum", [2, 1, G4], F16, kind="ExternalInput").ap()
    d["linb"] = nc.dram_tensor("linb", [2, 4, 128], F32, kind="ExternalInput").ap()
    d["ident8"] = nc.dram_tensor("ident8", [8, 8], F16, kind="ExternalInput").ap()
    d["ones120"] = nc.dram_tensor("ones120", [1, 120], F16, kind="ExternalInput").ap()
    d["ones60c"] = nc.dram_tensor("ones60c", [60, 1], F32R, kind="ExternalInput").ap()
    d["ones60r"] = nc.dram_tensor("ones60r", [1, 60], F32R, kind="ExternalInput").ap()
    d["x3T"] = nc.dram_tensor("x3T", [H, NT], F32, kind="ExternalOutput").ap()

    with tile.TileContext(nc) as tc:
        _k1_body(nc, tc, d)
    nc.compile()
    return nc


def _k1_body(nc, tc, d):
    import contextlib
    ctx = contextlib.ExitStack()
    with ctx:
        ctx.enter_context(nc.allow_low_precision(reason="fp16/f32r stores intended; fp32 accumulation"))
        const = ctx.enter_context(tc.tile_pool(name="const", bufs=1))
        wpool = ctx.enter_context(tc.tile_pool(name="wpool", bufs=2))
        xpool = ctx.enter_context(tc.tile_pool(name="xpool", bufs=1))
        ring_p = ctx.enter_context(tc.tile_pool(name="ringp", bufs=3))
        state = ctx.enter_context(tc.tile_pool(name="state", bufs=1))
        tmp = ctx.enter_context(tc.tile_pool(name="tmp", bufs=3))
        stage_p = ctx.enter_context(tc.tile_pool(name="stagep", bufs=2))
        psum = ctx.enter_context(tc.tile_pool(name="psum", bufs=2, space="PSUM"))
        psum_tp = ctx.enter_context(tc.tile_pool(name="psumtp", bufs=4, space="PSUM"))
        dram = ctx.enter_context(tc.tile_pool(name="dram", bufs=2, space="DRAM"))

        QOFF = (0, 512, 1536, 1024)  # quadrant -> gate column offset (i, f, o, g)
        ident8 = const.tile([8, 8], F16)
        nc.sync.dma_start(ident8[:], d["ident8"][:])
        identb = const.tile([128, 8], F16)
        nc.sync.dma_start(identb[64:72, :], d["ident8"][:])
        ones120 = const.tile([1, 120], F16)
        nc.sync.dma_start(ones120[:], d["ones120"][:])
        ones60c = const.tile([60, 1], F32R)
        nc.sync.dma_start(ones60c[:], d["ones60c"][:])
        ones60r = const.tile([1, 60], F32R)
        nc.sync.dma_start(ones60r[:], d["ones60r"][:])
        linb_sb = const.tile([128, 2, 4], F32)
        for l in range(2):
            for m in range(4):
                nc.sync.dma_start(linb_sb[:, l, m].unsqueeze(-1), d["linb"][l, m, :].unsqueeze(-1))
        scvec = const.tile([104, 1], F32)
        nc.vector.memset(scvec[:], 1.0)
        nc.vector.memset(scvec[96:104, :], 2.0)
        mask_sb = const.tile([1, NT], F16)
        nc.sync.dma_start(mask_sb[:], d["mask"][:])
        bsum_sb = const.tile([1, 2, G4], F16)
        for l in range(2):
            nc.sync.dma_start(bsum_sb[:, l, :], d["bsum"][l])

        x1T_sb = xpool.tile([128, 4, NT], F16, tag="x1T")
        encT_sb = xpool.tile([128, 4, NT], F16, tag="encT")
        for k in range(4):
            nc.sync.dma_start(x1T_sb[:, k, :], d["x1T"][k * 128:(k + 1) * 128, :])
            nc.sync.dma_start(encT_sb[:, k, :], d["encT"][k * 128:(k + 1) * 128, :])
        encs_sb = xpool.tile([60, BL, H], F32R, tag="encs")
        nc.sync.dma_start(encs_sb[:], d["enc_s"][:])
        x2T_sb = xpool.tile([128, 4, NT], F16, tag="x2T")
        catT0 = xpool.tile([128, T, 8, 8], F16, tag="catT0")
        catT1 = xpool.tile([128, T, 8, 8], F16, tag="catT1")
        catT = [catT0, catT1]

        xg_sb0 = xpool.tile([120, 4, G4], F16, tag="xg0")
        xg_sb1 = xpool.tile([120, 4, G4], F16, tag="xg1")
        xg_sb = [xg_sb0, xg_sb1]

        for l in range(2):
            xT_sb = x1T_sb if l == 0 else x2T_sb

            # -------- Phase A: xg = xT.T @ wihT[l] + bsum -> DRAM (t-major rows)
            wih_sb = wpool.tile([128, 4, G4], F16, tag="W")
            for k in range(4):
                nc.sync.dma_start(wih_sb[:, k, :], d["wihT"][l, k * 128:(k + 1) * 128, :])
            for m in range(4):
                for t_nn in range(4):
                    ps = psum.tile([128, 512], F32, tag="gates")
                    nc.tensor.matmul(ps[:120, :], ones120[:], bsum_sb[:, l, t_nn * 512:(t_nn + 1) * 512],
                                     start=True, stop=False)
                    for k in range(4):
                        nc.tensor.matmul(ps[:120, :], xT_sb[:, k, m * 120:(m + 1) * 120],
                                         wih_sb[:, k, t_nn * 512:(t_nn + 1) * 512],
                                         start=False, stop=(k == 3))
                    if t_nn % 2 == 0:
                        nc.vector.tensor_copy(xg_sb[l][:, m, t_nn * 512:(t_nn + 1) * 512], ps[:120, :])
                    else:
                        nc.scalar.copy(xg_sb[l][:, m, t_nn * 512:(t_nn + 1) * 512], ps[:120, :])

            # -------- Phase B: recurrence (4-quadrant packed gates: i,f,g,o)
            whh_sb = wpool.tile([128, 4, G4], F16, tag="W")
            for k in range(4):
                nc.sync.dma_start(whh_sb[:, k, :], d["whhT"][l, k * 128:(k + 1) * 128, :])
            c_sb = state.tile([8, H], F16, tag=f"c{l}")
            if l == 0:
                for _slot in range(2):
                    ginit = psum.tile([128, 512], F32, tag="gates")
                    nc.vector.memset(ginit[:], 0.0)
            for t in range(T):
                ring = ring_p.tile([8, G4], F16, tag="ring")
                nc.sync.dma_start(ring[:], xg_sb[l][(t % 15) * 8:(t % 15) * 8 + 8, t // 15, :])
                gates = psum.tile([128, 512], F32, tag="gates")
                for q in range(4):
                    nc.tensor.matmul(gates[32 * q:32 * q + 8, :], ident8[:],
                                     ring[:, QOFF[q]:QOFF[q] + 512],
                                     start=True, stop=(t == 0), tile_position=(0, 32 * q))
                    if t > 0:
                        for k in range(4):
                            nc.tensor.matmul(gates[32 * q:32 * q + 8, :],
                                             catT[l][:, (t - 1), k, :],
                                             whh_sb[:, k, QOFF[q]:QOFF[q] + 512],
                                             start=False, stop=(k == 3), tile_position=(0, 32 * q))
                sig_ps = psum.tile([128, 512], F32, tag="sig")
                nc.scalar.activation(sig_ps[0:104, :], gates[0:104, :], AF.Sigmoid, scale=scvec[:])
                o_sb = tmp.tile([128, 512], F16, tag="osb")
                nc.scalar.activation(o_sb[64:72, :], gates[64:72, :], AF.Sigmoid)
                tg_sb = tmp.tile([8, H], F16, tag="tg")
                nc.vector.tensor_scalar(tg_sb[:], sig_ps[96:104, :], 2.0, -1.0, op0=OP.mult, op1=OP.add)
                t2 = tmp.tile([8, H], F16, tag="t2")
                nc.vector.tensor_tensor(t2[:], sig_ps[0:8, :], tg_sb[:], op=OP.mult)
                if t == 0:
                    nc.vector.tensor_copy(c_sb[:], t2[:])
                else:
                    t1 = tmp.tile([8, H], F16, tag="t1")
                    nc.vector.tensor_tensor(t1[:], sig_ps[32:40, :], c_sb[:], op=OP.mult)
                    nc.vector.tensor_tensor(c_sb[:], t1[:], t2[:], op=OP.add)
                oT_ps = psum_tp.tile([128, 4, 8], F16, tag="tp")
                cT_ps = psum_tp.tile([128, 4, 8], F16, tag="tp")
                for k in range(4):
                    nc.tensor.transpose(oT_ps[:, k, :], o_sb[64:72, k * 128:(k + 1) * 128], identb[64:72, :])
                    nc.tensor.transpose(cT_ps[:, k, :], c_sb[:, k * 128:(k + 1) * 128], ident8[:])
                thcT = tmp.tile([128, 32], F16, tag="thcT")
                nc.scalar.activation(thcT[:], cT_ps[:].rearrange("p a b -> p (a b)"), AF.Tanh)
                nc.vector.tensor_tensor(catT[l][:, t, 0:4, :].rearrange("p a b -> p (a b)"),
                                        oT_ps[:].rearrange("p a b -> p (a b)"), thcT[:], op=OP.mult)

            # -------- Phase C: attention
            for b in range(BL):
                scT = psum.tile([128, 512], F32, tag="gates")
                nc.tensor.matmul(scT[:60, :60], mask_sb[:, b * 60:(b + 1) * 60], ones120[:, :60],
                                 start=True, stop=False)
                dec_b = catT[l].rearrange("p t c b -> p c b t")
                for k in range(4):
                    nc.tensor.matmul(scT[:60, :60], encT_sb[:, k, b * 60:(b + 1) * 60],
                                     dec_b[:, k, b, :], start=False, stop=(k == 3))
                E_sb = tmp.tile([60, 60], F32R, tag="E")
                nc.scalar.activation(E_sb[:], scT[:60, :60], AF.Exp)
                colsum = psum_tp.tile([1, 60], F32, tag="tp")
                nc.tensor.matmul(colsum[:], ones60c[:], E_sb[:], start=True, stop=True)
                r_sb = tmp.tile([1, 60], F32R, tag="r")
                nc.vector.reciprocal(r_sb[:], colsum[:])
                rbc = psum.tile([128, 512], F32, tag="sig")
                nc.tensor.matmul(rbc[:60, :60], ones60r[:], r_sb[:], start=True, stop=True)
                En_sb = tmp.tile([60, 60], F32R, tag="En")
                nc.vector.tensor_tensor(En_sb[:], E_sb[:], rbc[:60, :60], op=OP.mult)
                at_ps = psum_tp.tile([128, 4, 60], F32, tag="tp")
                cat_w = catT[l].rearrange("p t c b -> p c b t")
                for k in range(4):
                    nc.tensor.matmul(at_ps[:, k, :], encs_sb[:, b, k * 128:(k + 1) * 128], En_sb[:],
                                     start=True, stop=True)
                    nc.vector.tensor_copy(cat_w[:, 4 + k, b, :], at_ps[:, k, :])

            # -------- Phase D: lin
            lin_sb = wpool.tile([128, 8, H], F16, tag="W")
            for k2 in range(8):
                nc.sync.dma_start(lin_sb[:, k2, :], d["linT"][l, k2 * 128:(k2 + 1) * 128, :])
            cat_lin = catT[l].rearrange("p t c b -> p c t b")
            for m in range(4):
                ps = psum.tile([128, 512], F32, tag="gates")
                for k2 in range(8):
                    nc.tensor.matmul(ps[:, :NT].rearrange("p (t b) -> p t b", b=BL),
                                     lin_sb[:, k2, m * 128:(m + 1) * 128],
                                     cat_lin[:, k2, :, :], start=(k2 == 0), stop=(k2 == 7))
                if l == 0:
                    nc.scalar.activation(x2T_sb[:, m, :], ps[:, :NT], AF.Identity,
                                         bias=linb_sb[:, 0, m].unsqueeze(-1))
                else:
                    x3st = stage_p.tile([128, NT], F32, tag="stage")
                    nc.scalar.activation(x3st[:], ps[:, :NT], AF.Identity,
                                         bias=linb_sb[:, 1, m].unsqueeze(-1))
                    nc.sync.dma_start(d["x3T"][m * 128:(m + 1) * 128, :], x3st[:])


# ---------------------------------------------------------------------------
# kernel 2 builder (vocab-sharded projection)
# ---------------------------------------------------------------------------
def _build_k2():
    nc = bacc.Bacc("TRN2", target_bir_lowering=False, debug=False, num_devices=NCORES)
    x_d = nc.dram_tensor("x3T", [H, NBT], F16, kind="ExternalInput").ap()
    w_d = nc.dram_tensor("wT", [H, NV], F16, kind="ExternalInput").ap()
    ob_d = nc.dram_tensor("ob", [1, NV], F32R, kind="ExternalInput").ap()
    ones_d = nc.dram_tensor("ones128", [1, 128], F32R, kind="ExternalInput").ap()
    out_d = nc.dram_tensor("out", [NBT, NV], F32, kind="ExternalOutput").ap()

    with tile.TileContext(nc) as tc:
        import contextlib
        ctx = contextlib.ExitStack()
        with ctx:
            ctx.enter_context(nc.allow_low_precision(reason="fp16 inputs intended; fp32 accumulate"))
            pool = ctx.enter_context(tc.tile_pool(name="sb", bufs=1))
            stp = ctx.enter_context(tc.tile_pool(name="st", bufs=6))
            psum = ctx.enter_context(tc.tile_pool(name="ps", bufs=8, space="PSUM"))

            x_sb = pool.tile([128, 4, NBT], F16, tag="x")
            w_sb = pool.tile([128, 4, NV], F16, tag="w")
            for k in range(4):
                nc.sync.dma_start(x_sb[:, k, :], x_d[k * 128:(k + 1) * 128, :])
                nc.sync.dma_start(w_sb[:, k, :], w_d[k * 128:(k + 1) * 128, :])
            ob_sb = pool.tile([1, NV], F32R, tag="ob")
            nc.sync.dma_start(ob_sb[:], ob_d[:])
            ones_sb = pool.tile([1, 128], F32R, tag="ones")
            nc.sync.dma_start(ones_sb[:], ones_d[:])

            bias_sb = pool.tile([128, NV], F32, tag="bias")
            for n in range(NCH):
                bps = psum.tile([128, CH], F32, tag="pp")
                nc.tensor.matmul(bps[:], ones_sb[:], ob_sb[:, n * CH:(n + 1) * CH],
                                 start=True, stop=True)
                nc.vector.tensor_copy(bias_sb[:, n * CH:(n + 1) * CH], bps[:])

            for m in range(NBT // 128):
                pss = []
                for n in range(NCH):
                    ps = psum.tile([128, CH], F32, tag="pp")
                    pss.append(ps)
                for k in range(4):
                    for n in range(NCH):
                        nc.tensor.matmul(pss[n][:], x_sb[:, k, m * 128:(m + 1) * 128],
                                         w_sb[:, k, n * CH:(n + 1) * CH],
                                         start=(k == 0), stop=(k == 3))
                for n in range(NCH):
                    st = stp.tile([128, CH], F32, tag="st")
                    nc.vector.tensor_tensor(st[:], pss[n][:], bias_sb[:, n * CH:(n + 1) * CH], op=OP.add)
                    nc.sync.dma_start(out_d[m * 128:(m + 1) * 128, n * CH:(n + 1) * CH], st[:])
    nc.compile()
    return nc


# ---------------------------------------------------------------------------
# host side
# ---------------------------------------------------------------------------
_CACHE = {}


def _get_kernels():
    if "k1" not in _CACHE:
        _CACHE["k1"] = _build_k1()
        _CACHE["k2"] = _build_k2()
    return _CACHE["k1"], _CACHE["k2"]


def _host_prep_k1(inputs):
    f32, f16 = np.float32, np.float16
    indices = np.asarray(inputs["indices"]).astype(np.int64)
    emb = np.asarray(inputs["emb"], f32)
    enc = np.asarray(inputs["enc_output"], f32)
    de_lens = np.asarray(inputs["de_lens"]).astype(np.int64)
    w_ih = np.asarray(inputs["w_ih"], f32)
    w_hh = np.asarray(inputs["w_hh"], f32)
    bsum = np.asarray(inputs["b_ih"], f32) + np.asarray(inputs["b_hh"], f32)
    lin_w = np.asarray(inputs["lin_w"], f32)
    lin_b = np.asarray(inputs["lin_b"], f32)

    x1 = emb[indices]  # [B, T, H]
    mask = np.where(np.arange(S)[None, :] < de_lens[:, None], 0.0, NEG).astype(f32)

    wihT = np.ascontiguousarray(np.transpose(w_ih, (0, 2, 1))).astype(f16)
    whhT = np.ascontiguousarray(np.transpose(w_hh, (0, 2, 1))).astype(f16)
    linT = np.ascontiguousarray(np.transpose(lin_w, (0, 2, 1))).astype(f16)
    bsum = np.ascontiguousarray(bsum.reshape(L, 1, G4)).astype(f16)
    linb = np.ascontiguousarray(lin_b.reshape(L, 4, 128))

    consts = {
        "ident8": np.eye(8, dtype=f16),
        "ones120": np.ones((1, 120), f16),
        "ones60c": np.ones((60, 1), f32),
        "ones60r": np.ones((1, 60), f32),
    }
    in_maps = []
    for c in range(NCORES):
        bsl = slice(c * BL, (c + 1) * BL)
        x1c = x1[bsl]
        encc = enc[bsl]
        in_maps.append({
            "x1T": np.ascontiguousarray(np.transpose(x1c, (2, 1, 0)).reshape(H, NT)).astype(f16),
            "encT": np.ascontiguousarray(np.transpose(encc, (2, 0, 1)).reshape(H, BL * S)).astype(f16),
            "enc_s": np.ascontiguousarray(np.transpose(encc, (1, 0, 2))),
            "mask": np.ascontiguousarray(mask[bsl].reshape(1, BL * S)).astype(f16),
            "wihT": wihT, "whhT": whhT, "linT": linT, "bsum": bsum, "linb": linb,
            **consts,
        })
    return in_maps


def _run_spmd(nc, in_maps):
    import time as _time
    from concourse.bass_utils import run_bass_kernel_spmd
    last = None
    for attempt in range(4):
        try:
            return run_bass_kernel_spmd(nc, in_maps, core_ids=list(range(NCORES)))
        except Exception as e:  # transient NRT_EXEC_UNIT_UNRECOVERABLE etc.
            last = e
            _time.sleep(2.0 * (attempt + 1))
    raise last


def kernel(**inputs) -> np.ndarray:
    k1, k2 = _get_kernels()
    in_maps = _host_prep_k1(inputs)
    res1 = _run_spmd(k1, in_maps)

    # gather x3T: per-core [H, 480] t-major local -> global [H, B*T] b-major
    x3T = np.empty((H, NBT), np.float32)
    for c in range(NCORES):
        x3 = res1.results[c]["x3T"]
        for bl in range(BL):
            gb = c * BL + bl
            x3T[:, gb * T:(gb + 1) * T] = x3[:, bl::BL]
    x3T16 = x3T.astype(np.float16)

    out_w = np.asarray(inputs["out_w"], np.float32)
    out_b = np.asarray(inputs["out_b"], np.float32)
    ones128 = np.ones((1, 128), np.float32)
    k2maps = []
    for c in range(NCORES):
        vs = slice(c * NV, (c + 1) * NV)
        k2maps.append({
            "x3T": x3T16,
            "wT": np.ascontiguousarray(out_w[vs].T).astype(np.float16),
            "ob": np.ascontiguousarray(out_b[vs]).reshape(1, NV).astype(np.float32),
            "ones128": ones128,
        })
    res2 = _run_spmd(k2, k2maps)
    logits = np.concatenate([res2.results[c]["out"] for c in range(NCORES)], axis=1)
    return logits.reshape(B, T, V).astype(np.float32)



# revision 7
# speedup vs baseline: 2.4297x; 2.4297x over previous
"""Trainium2 Bass kernel for nn_DecoderAttention3 (2-layer LSTM decoder with
attention + vocab projection), distributed over 8 NeuronCores.

Single fused module, data-parallel over batch (8 batches/core):
  Phase A(l): xg = x @ W_ih.T + b  (bulk GEMM, t-major PSUM -> SBUF fp16)
  Recurrence(l): transposed-gates formulation. Gates live as [128 gate-part,
    16 chunks, 8 batch] in PSUM; per step the h @ W_hh.T contribution is 64
    matmuls with the small batch dim (N=8) as the moving operand and W_hh
    chunks as stationary, plus 16 tiny ident-matmuls injecting xg.
    Elementwise cell update runs on Act/DVE/Pool in [gate-part, batch] layout,
    so no transposes are needed and h lands directly in matmul-ready layout.
  Attention(l): bulk over all (b, t) with safe softmax (max-subtracted, fp16
    exp) in [t-part, s-free] orientation; E transposed back per-b via PE.
  lin(l): bulk GEMM -> x{2,3}T fp16.
  Projection: x3T @ out_w.T over the full vocab per core (batch-sharded),
    out_w streamed from DRAM in 1024-vocab chunks, fp16 logits to DRAM.
    Host adds out_b and casts to fp32.

Gate columns are host-permuted to [i, f, o, g] with the g block pre-scaled by
2 so one sigmoid covers all gates (tanh(x) = 2*sigmoid(2x) - 1).
"""
import sys
for p in ('/opt/trn_rl_repo', '/root/.axon_site/_ro/trn_rl_repo'):
    if p not in sys.path:
        sys.path.insert(0, p)

import contextlib

import numpy as np

import concourse.bass as bass
import concourse.tile as tile
from concourse import bacc, mybir

F32 = mybir.dt.float32
F16 = mybir.dt.float16
AF = mybir.ActivationFunctionType
OP = mybir.AluOpType

B, T, S, H, V, L = 64, 60, 60, 512, 32000, 2
NCORES = 8
BL = B // NCORES          # 8 local batches per core
Tp = 64                   # padded T (multiple of 16)
NT = T * BL               # 480
NTp = Tp * BL             # 512
G4 = 4 * H                # 2048
NGC = G4 // 128           # 16 gate chunks
VC = 1024                 # vocab chunk for streamed projection
NVC = V // VC             # 32
NEG = -30000.0


# ---------------------------------------------------------------------------
# kernel builder
# ---------------------------------------------------------------------------
def _build():
    nc = bacc.Bacc("TRN2", target_bir_lowering=False, debug=False, num_devices=NCORES)
    d = {}

    def inp(name, shape, dt=F16):
        d[name] = nc.dram_tensor(name, shape, dt, kind="ExternalInput").ap()

    inp("x1T", [H, NTp])
    inp("encT", [H, BL, S])
    inp("encs", [S, BL, H])
    inp("mask", [1, BL, S])
    inp("wihT", [L, H, G4])
    inp("whhT", [L, H, G4])
    inp("bsum", [L, 1, G4])
    inp("linT", [L, 2 * H, H])
    inp("linb", [L, 1, H])
    inp("wT", [H, V])
    inp("ones", [1, H])
    inp("ident8", [8, 8])
    inp("ident60", [S, S])
    inp("ones60c", [S, 1])
    d["out"] = nc.dram_tensor("out", [NT, V], F16, kind="ExternalOutput").ap()

    with tile.TileContext(nc) as tc:
        _body(nc, tc, d)
    nc.compile()
    return nc


def _body(nc, tc, d):
    ctx = contextlib.ExitStack()
    with ctx:
        ctx.enter_context(nc.allow_low_precision(
            reason="fp16 activations/weights intended; fp32 PSUM accumulation"))
        const = ctx.enter_context(tc.tile_pool(name="const", bufs=1))
        xpool = ctx.enter_context(tc.tile_pool(name="xpool", bufs=1))
        wpool = ctx.enter_context(tc.tile_pool(name="wpool", bufs=1))
        small = ctx.enter_context(tc.tile_pool(name="small", bufs=3))
        cpool = ctx.enter_context(tc.tile_pool(name="cpool", bufs=2))
        wstr = ctx.enter_context(tc.tile_pool(name="wstr", bufs=2))
        ostg = ctx.enter_context(tc.tile_pool(name="ostg", bufs=4))
        p_mm = ctx.enter_context(tc.tile_pool(name="pmm", bufs=2, space="PSUM"))

        # ---- consts ----
        ident8 = const.tile([8, 8], F16)
        nc.sync.dma_start(ident8[:], d["ident8"][:])
        ident60 = const.tile([S, S], F16)
        nc.sync.dma_start(ident60[:], d["ident60"][:])
        ones = const.tile([1, H], F16)
        nc.sync.dma_start(ones[:], d["ones"][:])
        ones60c = const.tile([S, 1], F16)
        nc.sync.dma_start(ones60c[:], d["ones60c"][:])
        mask_sb = const.tile([1, BL, S], F16)
        nc.sync.dma_start(mask_sb[:], d["mask"][:])
        bsum_sb = const.tile([1, L, G4], F16)
        for l in range(L):
            nc.sync.dma_start(bsum_sb[:, l, :], d["bsum"][l])
        linb_sb = const.tile([1, L, H], F16)
        for l in range(L):
            nc.sync.dma_start(linb_sb[:, l, :], d["linb"][l])

        # ---- big inputs ----
        x1T_sb = xpool.tile([128, 4, NTp], F16, tag="x1T")
        wih = [wpool.tile([128, 4, G4], F16, tag=f"wih{l}", name=f"wih{l}") for l in range(L)]
        whh = [wpool.tile([128, 4, G4], F16, tag=f"whh{l}", name=f"whh{l}") for l in range(L)]
        lin_sb = [wpool.tile([128, 8, H], F16, tag=f"lin{l}", name=f"lin{l}") for l in range(L)]
        for k in range(4):
            nc.sync.dma_start(x1T_sb[:, k, :], d["x1T"][k * 128:(k + 1) * 128, :])
        for k in range(4):
            nc.sync.dma_start(wih[0][:, k, :], d["wihT"][0, k * 128:(k + 1) * 128, :])
        for k in range(4):
            nc.sync.dma_start(whh[0][:, k, :], d["whhT"][0, k * 128:(k + 1) * 128, :])
        encT_sb = xpool.tile([128, 4, BL, S], F16, tag="encT")
        for k in range(4):
            nc.sync.dma_start(encT_sb[:, k, :, :], d["encT"][k * 128:(k + 1) * 128, :, :])
        encs_sb = xpool.tile([S, BL, H], F16, tag="encs")
        nc.sync.dma_start(encs_sb[:], d["encs"][:])
        for k in range(4):
            nc.sync.dma_start(wih[1][:, k, :], d["wihT"][1, k * 128:(k + 1) * 128, :])
        for k in range(4):
            nc.sync.dma_start(whh[1][:, k, :], d["whhT"][1, k * 128:(k + 1) * 128, :])
        for l in range(L):
            for k2 in range(8):
                nc.sync.dma_start(lin_sb[l][:, k2, :], d["linT"][l, k2 * 128:(k2 + 1) * 128, :])

        catT = [xpool.tile([128, 8, Tp, BL], F16, tag=f"catT{l}", name=f"catT{l}") for l in range(L)]
        x2T_sb = xpool.tile([128, 4, NTp], F16, tag="x2T")
        x3T_sb = xpool.tile([128, 4, NTp], F16, tag="x3T")

        for l in range(L):
            nc.vector.memset(catT[l][:, :, T:Tp, :], 0.0)

        # =================================================================
        # layers
        # =================================================================
        for l in range(L):
            xT = x1T_sb if l == 0 else x2T_sb

            # ---- Recurrence (transposed gates; xg computed in-step) ----
            # sigma chunk layout (after host permute): i=0:4, f=4:8, o=8:12, g=12:16
            rec_ctx = tc.tile_pool(name=f"pg{l}", bufs=2, space="PSUM")
            p_g = rec_ctx.__enter__()
            g_tiles = {}

            def emit_xg(t, l=l, xT=xT, g_tiles=g_tiles):
                # bias + x@Wih.T contribution for step t (independent of h)
                g = p_g.tile([128, NGC, BL], F32, tag="g", name="g")
                g_tiles[t] = g
                for gc in range(NGC):
                    nc.tensor.matmul(g[:, gc, :], bsum_sb[:, l, gc * 128:(gc + 1) * 128],
                                     ones[:, :BL], start=True, stop=False)
                    for k in range(4):
                        nc.tensor.matmul(g[:, gc, :],
                                         wih[l][:, k, gc * 128:(gc + 1) * 128],
                                         xT[:, k, t * BL:(t + 1) * BL],
                                         start=False, stop=(t == 0 and k == 3))

            emit_xg(0)
            for t in range(T):
                gates = g_tiles.pop(t)
                if t > 0:
                    for gc in range(NGC):
                        for k in range(4):
                            nc.tensor.matmul(gates[:, gc, :],
                                             whh[l][:, k, gc * 128:(gc + 1) * 128],
                                             catT[l][:, k, t - 1, :],
                                             start=False, stop=(k == 3))
                if t + 1 < T:
                    emit_xg(t + 1)
                sg = small.tile([128, NGC, BL], F16, tag="sg")
                nc.scalar.activation(sg[:], gates[:], AF.Sigmoid)
                tg = small.tile([128, 4, BL], F16, tag="tg")
                nc.vector.tensor_scalar(tg[:], sg[:, 12:16, :], 2.0, -1.0,
                                        op0=OP.mult, op1=OP.add)
                t2 = small.tile([128, 4, BL], F16, tag="t2")
                nc.vector.tensor_tensor(t2[:], sg[:, 0:4, :], tg[:], op=OP.mult)
                c_new = cpool.tile([128, 4, BL], F16, tag="c")
                if t == 0:
                    nc.vector.tensor_copy(c_new[:], t2[:])
                else:
                    t1 = small.tile([128, 4, BL], F16, tag="t1")
                    nc.gpsimd.tensor_tensor(t1[:], sg[:, 4:8, :], c_prev[:], op=OP.mult)
                    nc.vector.tensor_tensor(c_new[:], t1[:], t2[:], op=OP.add)
                c_prev = c_new
                thc = small.tile([128, 4, BL], F16, tag="thc")
                nc.scalar.activation(thc[:], c_new[:], AF.Tanh)
                nc.vector.tensor_tensor(catT[l][:, 0:4, t, :], sg[:, 8:12, :], thc[:],
                                        op=OP.mult)
            rec_ctx.__exit__(None, None, None)

            # ---- Attention (bulk, safe softmax) ----
            with tc.tile_pool(name=f"pat{l}", bufs=1, space="PSUM") as p_at, \
                 tc.tile_pool(name=f"par{l}", bufs=2, space="PSUM") as p_ar, \
                 tc.tile_pool(name=f"sat{l}", bufs=1) as s_at:
                scT = p_at.tile([S, BL, S], F32, tag="scT")
                for b in range(BL):
                    # scores[t, s] for batch b; mask broadcast over t rows
                    nc.tensor.matmul(scT[:, b, :], ones[:, :S], mask_sb[:, b, :],
                                     start=True, stop=False)
                    for k in range(4):
                        nc.tensor.matmul(scT[:, b, :],
                                         catT[l][:, k, 0:S, b],
                                         encT_sb[:, k, b, :],
                                         start=False, stop=(k == 3))
                mx = s_at.tile([S, BL], F32, tag="mx")
                nc.vector.tensor_reduce(mx[:], scT[:], mybir.AxisListType.X, OP.max)
                nmx = s_at.tile([S, BL], F32, tag="nmx")
                nc.vector.tensor_scalar_mul(nmx[:], mx[:], -1.0)
                E_sb = s_at.tile([S, BL, S], F16, tag="E")
                for b in range(BL):
                    nc.scalar.activation(E_sb[:, b, :], scT[:, b, :], AF.Exp,
                                         bias=nmx[:, b].unsqueeze(-1))
                ET_ps = p_at.tile([S, BL, S], F16, tag="ET")
                for b in range(BL):
                    nc.tensor.transpose(ET_ps[:, b, :], E_sb[:, b, :], ident60[:])
                ET_sb = s_at.tile([S, BL, S], F16, tag="ETs")
                nc.vector.tensor_copy(ET_sb[:], ET_ps[:])
                zr = p_at.tile([128, BL * S], F32, tag="zr")
                nc.tensor.matmul(zr[0:1, :], ones60c[:], ET_sb[:].rearrange("p a b -> p (a b)"),
                                 start=True, stop=True)
                r_sb = s_at.tile([1, BL * S], F16, tag="r")
                nc.vector.reciprocal(r_sb[:], zr[0:1, :])
                rbc = p_at.tile([128, BL * S], F32, tag="rbc")
                nc.tensor.matmul(rbc[:], ones[:, :128], r_sb[:], start=True, stop=True)
                rbc_sb = s_at.tile([128, BL, S], F16, tag="rbcs")
                nc.scalar.copy(rbc_sb[:].rearrange("p a b -> p (a b)"), rbc[:])
                for b in range(BL):
                    araw = p_ar.tile([128, 4, S], F32, tag="ar")
                    for k in range(4):
                        nc.tensor.matmul(araw[:, k, :],
                                         encs_sb[:, b, k * 128:(k + 1) * 128],
                                         ET_sb[:, b, :], start=True, stop=True)
                    eng = nc.vector if b % 2 == 0 else nc.gpsimd
                    eng.tensor_tensor(catT[l][:, 4:8, 0:S, b], araw[:],
                                      rbc_sb[:, b, :].unsqueeze(1).broadcast_to((128, 4, S)),
                                      op=OP.mult)

            # ---- lin: x_{l+1}T = linT[l].T @ catT[l] + linb[l] ----
            xoT = x2T_sb if l == 0 else x3T_sb
            for m in range(4):
                ps = p_mm.tile([128, 512], F32, tag="mm")
                nc.tensor.matmul(ps[:], linb_sb[:, l, m * 128:(m + 1) * 128],
                                 ones[:, :NTp], start=True, stop=False)
                for k2 in range(8):
                    nc.tensor.matmul(ps[:],
                                     lin_sb[l][:, k2, m * 128:(m + 1) * 128],
                                     catT[l][:, k2, :, :].rearrange("p a b -> p (a b)"),
                                     start=False, stop=(k2 == 7))
                if m % 2 == 0:
                    nc.scalar.copy(xoT[:, m, :], ps[:])
                else:
                    nc.vector.tensor_copy(xoT[:, m, :], ps[:])

        # =================================================================
        # projection: out[pos, v] = x3T.T @ wT   (fp16 out; host adds bias)
        # =================================================================
        for vc in range(NVC):
            wv = wstr.tile([128, 4, VC], F16, tag="wv")
            for k in range(4):
                nc.sync.dma_start(wv[:, k, :],
                                  d["wT"][k * 128:(k + 1) * 128, vc * VC:(vc + 1) * VC])
            for m in range(4):
                stg = ostg.tile([128, VC], F16, tag="st")
                for n in range(VC // 512):
                    ps = p_mm.tile([128, 512], F32, tag="mm")
                    for k in range(4):
                        nc.tensor.matmul(ps[:],
                                         x3T_sb[:, k, m * 128:(m + 1) * 128],
                                         wv[:, k, n * 512:(n + 1) * 512],
                                         start=(k == 0), stop=(k == 3))
                    eng = (nc.scalar.copy, nc.vector.tensor_copy)[(m * 2 + n) % 2]
                    eng(stg[:, n * 512:(n + 1) * 512], ps[:])
                rows = 128 if m < 3 else NT - 384
                nc.sync.dma_start(d["out"][m * 128:m * 128 + rows, vc * VC:(vc + 1) * VC],
                                  stg[:rows, :])


# ---------------------------------------------------------------------------
# host side
# ---------------------------------------------------------------------------
_CACHE = {}


def _get_modules():
    if "k" not in _CACHE:
        _CACHE["k"] = _build()
    return (_CACHE["k"],)


def _permute_gates(w):
    """[..., 4H] gate-major i,f,g,o -> [i, f, o, 2*g] (last axis)."""
    i, f, g, o = np.split(w, 4, axis=-1)
    return np.concatenate([i, f, o, 2.0 * g], axis=-1)


def _host_prep(inputs):
    f32, f16 = np.float32, np.float16
    indices = np.asarray(inputs["indices"]).astype(np.int64)
    emb = np.asarray(inputs["emb"], f32)
    enc = np.asarray(inputs["enc_output"], f32)
    de_lens = np.asarray(inputs["de_lens"]).astype(np.int64)
    w_ih = np.asarray(inputs["w_ih"], f32)
    w_hh = np.asarray(inputs["w_hh"], f32)
    bsum = np.asarray(inputs["b_ih"], f32) + np.asarray(inputs["b_hh"], f32)
    lin_w = np.asarray(inputs["lin_w"], f32)
    lin_b = np.asarray(inputs["lin_b"], f32)
    out_w = np.asarray(inputs["out_w"], f32)

    x1 = emb[indices]  # [B, T, H]
    mask = np.where(np.arange(S)[None, :] < de_lens[:, None], 0.0, NEG).astype(f32)

    # W [4H, H] -> W.T [H, 4H] with gate columns permuted to [i, f, o, 2g]
    wihT = _permute_gates(np.transpose(w_ih, (0, 2, 1))).astype(f16)
    whhT = _permute_gates(np.transpose(w_hh, (0, 2, 1))).astype(f16)
    bsum = _permute_gates(bsum).reshape(L, 1, G4).astype(f16)
    linT = np.ascontiguousarray(np.transpose(lin_w, (0, 2, 1))).astype(f16)
    linb = np.ascontiguousarray(lin_b.reshape(L, 1, H)).astype(f16)
    wT = np.ascontiguousarray(out_w.T).astype(f16)

    consts = {
        "wihT": np.ascontiguousarray(wihT), "whhT": np.ascontiguousarray(whhT),
        "bsum": bsum, "linT": linT, "linb": linb, "wT": wT,
        "ones": np.ones((1, H), f16),
        "ident8": np.eye(8, dtype=f16),
        "ident60": np.eye(S, dtype=f16),
        "ones60c": np.ones((S, 1), f16),
    }
    in_maps = []
    for c in range(NCORES):
        bsl = slice(c * BL, (c + 1) * BL)
        x1c = x1[bsl]                      # [BL, T, H]
        encc = enc[bsl]                    # [BL, S, H]
        x1T = np.zeros((H, NTp), f16)
        x1T[:, :NT] = np.transpose(x1c, (2, 1, 0)).reshape(H, NT)
        in_maps.append({
            "x1T": x1T,
            "encT": np.ascontiguousarray(np.transpose(encc, (2, 0, 1))).astype(f16),
            "encs": np.ascontiguousarray(np.transpose(encc, (1, 0, 2))).astype(f16),
            "mask": np.ascontiguousarray(mask[bsl].reshape(1, BL, S)).astype(f16),
            **consts,
        })
    return in_maps


def _run_spmd(nc, in_maps):
    import time as _time
    from concourse.bass_utils import run_bass_kernel_spmd
    last = None
    for attempt in range(4):
        try:
            return run_bass_kernel_spmd(nc, in_maps, core_ids=list(range(NCORES)))
        except Exception as e:  # transient NRT_EXEC_UNIT_UNRECOVERABLE etc.
            last = e
            _time.sleep(2.0 * (attempt + 1))
    raise last


def kernel(**inputs) -> np.ndarray:
    (k,) = _get_modules()
    in_maps = _host_prep(inputs)
    res = _run_spmd(k, in_maps)
    out_b = np.asarray(inputs["out_b"], np.float32)
    logits = np.empty((B, T, V), np.float32)
    for c in range(NCORES):
        oc = res.results[c]["out"].astype(np.float32)      # [NT, V] t-major
        logits[c * BL:(c + 1) * BL] = oc.reshape(T, BL, V).transpose(1, 0, 2)
    logits += out_b
    return logits


# revision 9
# speedup vs baseline: 2.4654x; 1.0147x over previous
"""Trainium2 Bass kernel for nn_DecoderAttention3 (2-layer LSTM decoder with
attention + vocab projection), distributed over 8 NeuronCores.

Single fused module, data-parallel over batch (8 batches/core):
  Phase A(l): xg = x @ W_ih.T + b  (bulk GEMM, t-major PSUM -> SBUF fp16)
  Recurrence(l): transposed-gates formulation. Gates live as [128 gate-part,
    16 chunks, 8 batch] in PSUM; per step the h @ W_hh.T contribution is 64
    matmuls with the small batch dim (N=8) as the moving operand and W_hh
    chunks as stationary, plus 16 tiny ident-matmuls injecting xg.
    Elementwise cell update runs on Act/DVE/Pool in [gate-part, batch] layout,
    so no transposes are needed and h lands directly in matmul-ready layout.
  Attention(l): bulk over all (b, t) with safe softmax (max-subtracted, fp16
    exp) in [t-part, s-free] orientation; E transposed back per-b via PE.
  lin(l): bulk GEMM -> x{2,3}T fp16.
  Projection: x3T @ out_w.T over the full vocab per core (batch-sharded),
    out_w streamed from DRAM in 1024-vocab chunks, fp16 logits to DRAM.
    Host adds out_b and casts to fp32.

Gate columns are host-permuted to [i, f, o, g] with the g block pre-scaled by
2 so one sigmoid covers all gates (tanh(x) = 2*sigmoid(2x) - 1).
"""
import sys
for p in ('/opt/trn_rl_repo', '/root/.axon_site/_ro/trn_rl_repo'):
    if p not in sys.path:
        sys.path.insert(0, p)

import contextlib

import numpy as np

import concourse.bass as bass
import concourse.tile as tile
from concourse import bacc, mybir

F32 = mybir.dt.float32
F16 = mybir.dt.float16
AF = mybir.ActivationFunctionType
OP = mybir.AluOpType

B, T, S, H, V, L = 64, 60, 60, 512, 32000, 2
NCORES = 8
BL = B // NCORES          # 8 local batches per core
Tp = 64                   # padded T (multiple of 16)
NT = T * BL               # 480
NTp = Tp * BL             # 512
G4 = 4 * H                # 2048
NGC = G4 // 128           # 16 gate chunks
VC = 1024                 # vocab chunk for streamed projection
NVC = V // VC             # 32
NEG = -30000.0


# ---------------------------------------------------------------------------
# kernel builder
# ---------------------------------------------------------------------------
def _build():
    nc = bacc.Bacc("TRN2", target_bir_lowering=False, debug=False, num_devices=NCORES)
    d = {}

    def inp(name, shape, dt=F16):
        d[name] = nc.dram_tensor(name, shape, dt, kind="ExternalInput").ap()

    inp("x1T", [H, NTp])
    inp("encT", [H, BL, S])
    inp("encs", [S, BL, H])
    inp("mask", [1, BL, S])
    inp("wihT", [L, H, G4])
    inp("whhT", [L, H, G4])
    inp("bsum", [L, 1, G4])
    inp("linT", [L, 2 * H, H])
    inp("linb", [L, 1, H])
    inp("wT", [H, V])
    inp("ones", [1, H])
    inp("ident8", [8, 8])
    inp("ident60", [S, S])
    inp("ones60c", [S, 1])
    d["out"] = nc.dram_tensor("out", [NT, V], F16, kind="ExternalOutput").ap()

    with tile.TileContext(nc) as tc:
        _body(nc, tc, d)
    nc.compile()
    return nc


def _body(nc, tc, d):
    ctx = contextlib.ExitStack()
    with ctx:
        ctx.enter_context(nc.allow_low_precision(
            reason="fp16 activations/weights intended; fp32 PSUM accumulation"))
        const = ctx.enter_context(tc.tile_pool(name="const", bufs=1))
        xpool = ctx.enter_context(tc.tile_pool(name="xpool", bufs=1))
        wpool = ctx.enter_context(tc.tile_pool(name="wpool", bufs=1))
        small = ctx.enter_context(tc.tile_pool(name="small", bufs=3))
        cpool = ctx.enter_context(tc.tile_pool(name="cpool", bufs=2))
        wstr = ctx.enter_context(tc.tile_pool(name="wstr", bufs=2))
        ostg = ctx.enter_context(tc.tile_pool(name="ostg", bufs=4))
        p_mm = ctx.enter_context(tc.tile_pool(name="pmm", bufs=2, space="PSUM"))

        # ---- consts ----
        ident8 = const.tile([8, 8], F16)
        nc.sync.dma_start(ident8[:], d["ident8"][:])
        ident60 = const.tile([S, S], F16)
        nc.sync.dma_start(ident60[:], d["ident60"][:])
        ones = const.tile([1, H], F16)
        nc.sync.dma_start(ones[:], d["ones"][:])
        ones60c = const.tile([S, 1], F16)
        nc.sync.dma_start(ones60c[:], d["ones60c"][:])
        mask_sb = const.tile([1, BL, S], F16)
        nc.sync.dma_start(mask_sb[:], d["mask"][:])
        bsum_sb = const.tile([1, L, G4], F16)
        for l in range(L):
            nc.sync.dma_start(bsum_sb[:, l, :], d["bsum"][l])
        linb_sb = const.tile([1, L, H], F16)
        for l in range(L):
            nc.sync.dma_start(linb_sb[:, l, :], d["linb"][l])

        # ---- big inputs ----
        x1T_sb = xpool.tile([128, 4, NTp], F16, tag="x1T")
        wih = [wpool.tile([128, 4, G4], F16, tag=f"wih{l}", name=f"wih{l}") for l in range(L)]
        whh = [wpool.tile([128, 4, G4], F16, tag=f"whh{l}", name=f"whh{l}") for l in range(L)]
        lin_sb = [wpool.tile([128, 8, H], F16, tag=f"lin{l}", name=f"lin{l}") for l in range(L)]
        for k in range(4):
            nc.sync.dma_start(x1T_sb[:, k, :], d["x1T"][k * 128:(k + 1) * 128, :])
        for k in range(4):
            nc.sync.dma_start(wih[0][:, k, :], d["wihT"][0, k * 128:(k + 1) * 128, :])
        for k in range(4):
            nc.sync.dma_start(whh[0][:, k, :], d["whhT"][0, k * 128:(k + 1) * 128, :])
        encT_sb = xpool.tile([128, 4, BL, S], F16, tag="encT")
        for k in range(4):
            nc.sync.dma_start(encT_sb[:, k, :, :], d["encT"][k * 128:(k + 1) * 128, :, :])
        encs_sb = xpool.tile([S, BL, H], F16, tag="encs")
        nc.sync.dma_start(encs_sb[:], d["encs"][:])
        for k in range(4):
            nc.sync.dma_start(wih[1][:, k, :], d["wihT"][1, k * 128:(k + 1) * 128, :])
        for k in range(4):
            nc.sync.dma_start(whh[1][:, k, :], d["whhT"][1, k * 128:(k + 1) * 128, :])
        for l in range(L):
            for k2 in range(8):
                nc.sync.dma_start(lin_sb[l][:, k2, :], d["linT"][l, k2 * 128:(k2 + 1) * 128, :])

        catT = [xpool.tile([128, 8, Tp, BL], F16, tag=f"catT{l}", name=f"catT{l}") for l in range(L)]
        x2T_sb = xpool.tile([128, 4, NTp], F16, tag="x2T")
        x3T_sb = xpool.tile([128, 4, NTp], F16, tag="x3T")

        for l in range(L):
            nc.vector.memset(catT[l][:, :, T:Tp, :], 0.0)

        # =================================================================
        # layers
        # =================================================================
        for l in range(L):
            xT = x1T_sb if l == 0 else x2T_sb

            # ---- Recurrence (transposed gates; xg batched 4 steps/matmul) ----
            # sigma chunk layout (after host permute): i=0:4, f=4:8, o=8:12, g=12:16
            rec_ctx = tc.tile_pool(name=f"pg{l}", bufs=2, space="PSUM")
            p_g = rec_ctx.__enter__()
            NSB = 4
            g_groups = {}

            def emit_xg_group(j, l=l, xT=xT, g_groups=g_groups):
                # bias + x@Wih.T contribution for steps 4j..4j+3 (h-independent)
                g4 = p_g.tile([128, NGC, NSB, BL], F32, tag="g", name="g")
                g_groups[j] = g4
                t0 = j * NSB
                for gc in range(NGC):
                    nc.tensor.matmul(g4[:, gc, :, :], bsum_sb[:, l, gc * 128:(gc + 1) * 128],
                                     ones[:, :NSB * BL], start=True, stop=False)
                    for k in range(4):
                        nc.tensor.matmul(g4[:, gc, :, :],
                                         wih[l][:, k, gc * 128:(gc + 1) * 128],
                                         xT[:, k, t0 * BL:(t0 + NSB) * BL],
                                         start=False, stop=(k == 3),
                                         skip_group_check=True)

            emit_xg_group(0)
            for t in range(T):
                j, r = divmod(t, NSB)
                g4 = g_groups[j]
                if r == 2 and j + 1 < T // NSB:
                    emit_xg_group(j + 1)
                if t > 0:
                    for gc in range(NGC):
                        for k in range(4):
                            nc.tensor.matmul(g4[:, gc, r, :],
                                             whh[l][:, k, gc * 128:(gc + 1) * 128],
                                             catT[l][:, k, t - 1, :],
                                             start=False, stop=(k == 3),
                                             skip_group_check=True)
                gates = g4[:, :, r, :]
                sg = small.tile([128, NGC, BL], F16, tag="sg")
                nc.scalar.activation(sg[:], gates[:], AF.Sigmoid)
                tg = small.tile([128, 4, BL], F16, tag="tg")
                nc.vector.tensor_scalar(tg[:], sg[:, 12:16, :], 2.0, -1.0,
                                        op0=OP.mult, op1=OP.add)
                t2 = small.tile([128, 4, BL], F16, tag="t2")
                nc.vector.tensor_tensor(t2[:], sg[:, 0:4, :], tg[:], op=OP.mult)
                c_new = cpool.tile([128, 4, BL], F16, tag="c")
                if t == 0:
                    nc.vector.tensor_copy(c_new[:], t2[:])
                else:
                    t1 = small.tile([128, 4, BL], F16, tag="t1")
                    nc.gpsimd.tensor_tensor(t1[:], sg[:, 4:8, :], c_prev[:], op=OP.mult)
                    nc.vector.tensor_tensor(c_new[:], t1[:], t2[:], op=OP.add)
                c_prev = c_new
                thc = small.tile([128, 4, BL], F16, tag="thc")
                nc.scalar.activation(thc[:], c_new[:], AF.Tanh)
                nc.vector.tensor_tensor(catT[l][:, 0:4, t, :], sg[:, 8:12, :], thc[:],
                                        op=OP.mult)
            rec_ctx.__exit__(None, None, None)

            # ---- Attention (bulk, safe softmax) ----
            with tc.tile_pool(name=f"pat{l}", bufs=1, space="PSUM") as p_at, \
                 tc.tile_pool(name=f"par{l}", bufs=2, space="PSUM") as p_ar, \
                 tc.tile_pool(name=f"sat{l}", bufs=1) as s_at:
                scT = p_at.tile([S, BL, S], F32, tag="scT")
                for b in range(BL):
                    # scores[t, s] for batch b; mask broadcast over t rows
                    nc.tensor.matmul(scT[:, b, :], ones[:, :S], mask_sb[:, b, :],
                                     start=True, stop=False)
                    for k in range(4):
                        nc.tensor.matmul(scT[:, b, :],
                                         catT[l][:, k, 0:S, b],
                                         encT_sb[:, k, b, :],
                                         start=False, stop=(k == 3))
                mx = s_at.tile([S, BL], F32, tag="mx")
                nc.vector.tensor_reduce(mx[:], scT[:], mybir.AxisListType.X, OP.max)
                nmx = s_at.tile([S, BL], F32, tag="nmx")
                nc.vector.tensor_scalar_mul(nmx[:], mx[:], -1.0)
                E_sb = s_at.tile([S, BL, S], F16, tag="E")
                for b in range(BL):
                    nc.scalar.activation(E_sb[:, b, :], scT[:, b, :], AF.Exp,
                                         bias=nmx[:, b].unsqueeze(-1))
                ET_ps = p_at.tile([S, BL, S], F16, tag="ET")
                for b in range(BL):
                    nc.tensor.transpose(ET_ps[:, b, :], E_sb[:, b, :], ident60[:])
                ET_sb = s_at.tile([S, BL, S], F16, tag="ETs")
                nc.vector.tensor_copy(ET_sb[:], ET_ps[:])
                zr = p_at.tile([128, BL * S], F32, tag="zr")
                nc.tensor.matmul(zr[0:1, :], ones60c[:], ET_sb[:].rearrange("p a b -> p (a b)"),
                                 start=True, stop=True)
                r_sb = s_at.tile([1, BL * S], F16, tag="r")
                nc.vector.reciprocal(r_sb[:], zr[0:1, :])
                rbc = p_at.tile([128, BL * S], F32, tag="rbc")
                nc.tensor.matmul(rbc[:], ones[:, :128], r_sb[:], start=True, stop=True)
                rbc_sb = s_at.tile([128, BL, S], F16, tag="rbcs")
                nc.scalar.copy(rbc_sb[:].rearrange("p a b -> p (a b)"), rbc[:])
                for b in range(BL):
                    araw = p_ar.tile([128, 4, S], F32, tag="ar")
                    for k in range(4):
                        nc.tensor.matmul(araw[:, k, :],
                                         encs_sb[:, b, k * 128:(k + 1) * 128],
                                         ET_sb[:, b, :], start=True, stop=True)
                    nc.vector.tensor_tensor(catT[l][:, 4:8, 0:S, b], araw[:],
                                            rbc_sb[:, b, :].unsqueeze(1).broadcast_to((128, 4, S)),
                                            op=OP.mult)

            # ---- lin: x_{l+1}T = linT[l].T @ catT[l] + linb[l] ----
            xoT = x2T_sb if l == 0 else x3T_sb
            for m in range(4):
                ps = p_mm.tile([128, 512], F32, tag="mm")
                nc.tensor.matmul(ps[:], linb_sb[:, l, m * 128:(m + 1) * 128],
                                 ones[:, :NTp], start=True, stop=False)
                for k2 in range(8):
                    nc.tensor.matmul(ps[:],
                                     lin_sb[l][:, k2, m * 128:(m + 1) * 128],
                                     catT[l][:, k2, :, :].rearrange("p a b -> p (a b)"),
                                     start=False, stop=(k2 == 7))
                if m % 2 == 0:
                    nc.scalar.copy(xoT[:, m, :], ps[:])
                else:
                    nc.vector.tensor_copy(xoT[:, m, :], ps[:])

        # =================================================================
        # projection: out[pos, v] = x3T.T @ wT   (fp16 out; host adds bias)
        # =================================================================
        for vc in range(NVC):
            wv = wstr.tile([128, 4, VC], F16, tag="wv")
            for k in range(4):
                nc.sync.dma_start(wv[:, k, :],
                                  d["wT"][k * 128:(k + 1) * 128, vc * VC:(vc + 1) * VC])
            for m in range(4):
                stg = ostg.tile([128, VC], F16, tag="st")
                for n in range(VC // 512):
                    ps = p_mm.tile([128, 512], F32, tag="mm")
                    for k in range(4):
                        nc.tensor.matmul(ps[:],
                                         x3T_sb[:, k, m * 128:(m + 1) * 128],
                                         wv[:, k, n * 512:(n + 1) * 512],
                                         start=(k == 0), stop=(k == 3))
                    eng = (nc.scalar.copy, nc.vector.tensor_copy)[(m * 2 + n) % 2]
                    eng(stg[:, n * 512:(n + 1) * 512], ps[:])
                rows = 128 if m < 3 else NT - 384
                nc.sync.dma_start(d["out"][m * 128:m * 128 + rows, vc * VC:(vc + 1) * VC],
                                  stg[:rows, :])


# ---------------------------------------------------------------------------
# host side
# ---------------------------------------------------------------------------
_CACHE = {}


def _get_modules():
    if "k" not in _CACHE:
        _CACHE["k"] = _build()
    return (_CACHE["k"],)


def _permute_gates(w):
    """[..., 4H] gate-major i,f,g,o -> [i, f, o, 2*g] (last axis)."""
    i, f, g, o = np.split(w, 4, axis=-1)
    return np.concatenate([i, f, o, 2.0 * g], axis=-1)


def _host_prep(inputs):
    f32, f16 = np.float32, np.float16
    indices = np.asarray(inputs["indices"]).astype(np.int64)
    emb = np.asarray(inputs["emb"], f32)
    enc = np.asarray(inputs["enc_output"], f32)
    de_lens = np.asarray(inputs["de_lens"]).astype(np.int64)
    w_ih = np.asarray(inputs["w_ih"], f32)
    w_hh = np.asarray(inputs["w_hh"], f32)
    bsum = np.asarray(inputs["b_ih"], f32) + np.asarray(inputs["b_hh"], f32)
    lin_w = np.asarray(inputs["lin_w"], f32)
    lin_b = np.asarray(inputs["lin_b"], f32)
    out_w = np.asarray(inputs["out_w"], f32)

    x1 = emb[indices]  # [B, T, H]
    mask = np.where(np.arange(S)[None, :] < de_lens[:, None], 0.0, NEG).astype(f32)

    # W [4H, H] -> W.T [H, 4H] with gate columns permuted to [i, f, o, 2g]
    wihT = _permute_gates(np.transpose(w_ih, (0, 2, 1))).astype(f16)
    whhT = _permute_gates(np.transpose(w_hh, (0, 2, 1))).astype(f16)
    bsum = _permute_gates(bsum).reshape(L, 1, G4).astype(f16)
    linT = np.ascontiguousarray(np.transpose(lin_w, (0, 2, 1))).astype(f16)
    linb = np.ascontiguousarray(lin_b.reshape(L, 1, H)).astype(f16)
    wT = np.ascontiguousarray(out_w.T).astype(f16)

    consts = {
        "wihT": np.ascontiguousarray(wihT), "whhT": np.ascontiguousarray(whhT),
        "bsum": bsum, "linT": linT, "linb": linb, "wT": wT,
        "ones": np.ones((1, H), f16),
        "ident8": np.eye(8, dtype=f16),
        "ident60": np.eye(S, dtype=f16),
        "ones60c": np.ones((S, 1), f16),
    }
    in_maps = []
    for c in range(NCORES):
        bsl = slice(c * BL, (c + 1) * BL)
        x1c = x1[bsl]                      # [BL, T, H]
        encc = enc[bsl]                    # [BL, S, H]
        x1T = np.zeros((H, NTp), f16)
        x1T[:, :NT] = np.transpose(x1c, (2, 1, 0)).reshape(H, NT)
        in_maps.append({
            "x1T": x1T,
            "encT": np.ascontiguousarray(np.transpose(encc, (2, 0, 1))).astype(f16),
            "encs": np.ascontiguousarray(np.transpose(encc, (1, 0, 2))).astype(f16),
            "mask": np.ascontiguousarray(mask[bsl].reshape(1, BL, S)).astype(f16),
            **consts,
        })
    return in_maps


def _run_spmd(nc, in_maps):
    import time as _time
    from concourse.bass_utils import run_bass_kernel_spmd
    last = None
    for attempt in range(4):
        try:
            return run_bass_kernel_spmd(nc, in_maps, core_ids=list(range(NCORES)))
        except Exception as e:  # transient NRT_EXEC_UNIT_UNRECOVERABLE etc.
            last = e
            _time.sleep(2.0 * (attempt + 1))
    raise last


def kernel(**inputs) -> np.ndarray:
    (k,) = _get_modules()
    in_maps = _host_prep(inputs)
    res = _run_spmd(k, in_maps)
    out_b = np.asarray(inputs["out_b"], np.float32)
    logits = np.empty((B, T, V), np.float32)
    for c in range(NCORES):
        oc = res.results[c]["out"].astype(np.float32)      # [NT, V] t-major
        logits[c * BL:(c + 1) * BL] = oc.reshape(T, BL, V).transpose(1, 0, 2)
    logits += out_b
    return logits
